# revision 15
# baseline (speedup 1.0000x reference)
"""Additive (Bahdanau) attention TRN2 kernel — 8 NeuronCores, data-parallel.

Math (per batch b):
    qh = queries[b] @ Wq   (Q, H);  kh = keys[b] @ Wk   (KV, H)
    scores[q,k] = sum_h wv[h] * tanh(qh[q,h] + kh[k,h])
    out = softmax(mask(scores)) @ values[b]

Key trick: tanh(x) ~= sum_j c_j sin(om_j x) on [-7, 7] (R=8 harmonics,
sup err ~1e-2, rel contribution ~4e-3), and sin separates over x = a + b:
    sin(om(a+b)) = sin(om a)cos(om b) + cos(om a)sin(om b)
so the (Q, KV, H) tanh tensor is never materialized. Per frequency the
kernel computes sin/cos of the SMALL projected tensors (ACT engine, with
DVE range-reduction into [-pi, pi] via one fused mul pass + at most one
add_range_wrap), then reduces over h with 2R accumulated PE matmuls into
a (64, KV) PSUM scores tile (wv and c_j folded into the q-side weights).
cos is obtained as -cos(w) = sin(|w| - pi/2) so one big ACT Sin per
frequency covers both halves. Projections are clamped to +-3.5 (clamp
error ~1e-4) so one wrap suffices at om_max = 2.69.

Softmax without max-subtraction (scores are O(1)): p = Exp(scores+mask)
with the row sum taken by the same ACT instruction (accum_out); masked
columns give exp(-1e6) = 0. Row normalization is folded into the output
copy. valid_lens sparsity: ki chunks of 128 beyond ceil(valid/128) are
skipped at compile time (per batch-slot, slot-max across cores; batches
sorted so heavy ones share a slot).
"""

import os
import sys

for _p in ("/opt/trn_rl_repo",):
    if os.path.isdir(_p) and _p not in sys.path:
        sys.path.insert(0, _p)

import numpy as np
import ml_dtypes

from concourse import bacc, bass, mybir, tile
from concourse.bass_utils import run_bass_kernel_spmd

BF = ml_dtypes.bfloat16
DT = mybir.dt
AFT = mybir.ActivationFunctionType
ALU = mybir.AluOpType

B, Q, KV, QS, H, DV = 16, 64, 512, 256, 256, 256
NCORES = 8
SLOTS = B // NCORES  # 2 batches per core

CLAMP = 3.5
NFREQ = 8
_L = 9.33
OMEGA = (np.arange(1, NFREQ + 1) * np.pi / _L).astype(np.float64)
_xg = np.linspace(-2 * CLAMP, 2 * CLAMP, 8001)
_A = np.sin(np.outer(_xg, OMEGA))
COEF, *_ = np.linalg.lstsq(_A, np.tanh(_xg), rcond=None)
PI = float(np.pi)
TWO_PI = float(2 * np.pi)
HALF_PI = float(np.pi / 2)

_BUILD_CACHE: dict = {}
LAST_RESULT = None  # BassKernelResults of the most recent run (for test.py)


def _build(nch_slots: tuple) -> "bacc.Bacc":
    nc = bacc.Bacc("TRN2", target_bir_lowering=False, debug=False)

    qT_d = nc.declare_dram_parameter("qT", [SLOTS, QS, Q], DT.bfloat16, isOutput=False)
    kT_d = nc.declare_dram_parameter("kT", [SLOTS, QS, KV], DT.bfloat16, isOutput=False)
    vv_d = nc.declare_dram_parameter("vv", [SLOTS, KV, DV], DT.bfloat16, isOutput=False)
    mk_d = nc.declare_dram_parameter("mk", [SLOTS, Q, KV], DT.float32, isOutput=False)
    wq_d = nc.declare_dram_parameter("wq", [QS, H], DT.bfloat16, isOutput=False)
    wk_d = nc.declare_dram_parameter("wk", [QS, H], DT.bfloat16, isOutput=False)
    wvc_d = nc.declare_dram_parameter("wvc", [128, 2, NFREQ], DT.float32, isOutput=False)
    id_d = nc.declare_dram_parameter("iden", [Q, Q], DT.bfloat16, isOutput=False)
    out_d = nc.declare_dram_parameter("out", [SLOTS, Q, DV], DT.float32, isOutput=True)

    with tile.TileContext(nc) as tc:
        with (
            tc.tile_pool(name="const", bufs=1) as constp,
            tc.tile_pool(name="io", bufs=2) as iop,
            tc.tile_pool(name="work", bufs=3) as workp,
            tc.tile_pool(name="sm", bufs=2) as smp,
            tc.tile_pool(name="ps", bufs=2, space="PSUM") as psp,
        ):
            wq_sb = constp.tile([128, 2, H], DT.bfloat16, name="wq_sb")
            nc.sync.dma_start(wq_sb[:], wq_d.ap().rearrange("(c p) h -> p c h", p=128))
            wk_sb = constp.tile([128, 2, H], DT.bfloat16, name="wk_sb")
            nc.sync.dma_start(wk_sb[:], wk_d.ap().rearrange("(c p) h -> p c h", p=128))
            wvc_sb = constp.tile([128, 2, NFREQ], DT.float32, name="wvc_sb")
            nc.sync.dma_start(wvc_sb[:], wvc_d.ap())
            iden_sb = constp.tile([Q, Q], DT.bfloat16, name="iden_sb")
            nc.sync.dma_start(iden_sb[:], id_d.ap())

            # Pre-warm the sin activation table while input DMAs run.
            _salt = os.environ.get("KERNEL_SALT", "")
            warm = constp.tile([1, 8], DT.float32, name=f"warm{_salt}")
            warm2 = constp.tile([1, 8], DT.float32, name="warm2")
            nc.vector.memset(warm[:], 0.0)
            nc.scalar.activation(warm2[:], warm[:], AFT.Sin)
            nhpi = constp.tile([128, 1], DT.float32, name="nhpi")
            nc.vector.memset(nhpi[:], -HALF_PI)

            slot_state = []
            io_state = []
            # ---- Loads + projections + clamps for both slots ----
            for s in range(SLOTS):
                nch = nch_slots[s]
                W = nch * 128

                qT_sb = iop.tile([128, 2, Q], DT.bfloat16, tag="qT", name="qT_sb")
                nc.scalar.dma_start(
                    qT_sb[:], qT_d[s].rearrange("(c p) q -> p c q", p=128)
                )
                kT_sb = iop.tile([128, 2, W], DT.bfloat16, tag="kT", name="kT_sb")
                for ksc in range(2):
                    nc.sync.dma_start(
                        kT_sb[:, ksc, :],
                        kT_d[s].rearrange("(c p) k -> p c k", p=128)[:, ksc, :W],
                    )
                v_sb = iop.tile([128, nch, DV], DT.bfloat16, tag="v", name="v_sb")
                nc.scalar.dma_start(
                    v_sb[:], vv_d[s].rearrange("(c p) d -> p c d", p=128)[:, :nch]
                )
                mk_sb = iop.tile([Q, W], DT.float32, tag="mk", name="mk_sb")
                nc.gpsimd.dma_start(mk_sb[:], mk_d[s][:, :W])

                # Projections (PSUM f32) then clamp to +-CLAMP into SBUF f32.
                qc = iop.tile([128, 2, Q], DT.float32, tag="qc", name="qc")
                kc = iop.tile([128, 2, W], DT.float32, tag="kc", name="kc")
                for hc in range(2):
                    hsl = slice(hc * 128, (hc + 1) * 128)
                    psq = psp.tile([128, Q], DT.float32, tag="pproj", name="psq")
                    for ksc in range(2):
                        nc.tensor.matmul(
                            psq[:],
                            wq_sb[:, ksc, hsl],
                            qT_sb[:, ksc, :],
                            start=(ksc == 0),
                            stop=(ksc == 1),
                        )
                    nc.vector.tensor_scalar(
                        qc[:, hc, :], psq[:], -CLAMP, CLAMP, ALU.max, ALU.min
                    )
                    psk = psp.tile([128, W], DT.float32, tag="pproj", name="psk")
                    for ksc in range(2):
                        nc.tensor.matmul(
                            psk[:],
                            wk_sb[:, ksc, hsl],
                            kT_sb[:, ksc, :],
                            start=(ksc == 0),
                            stop=(ksc == 1),
                        )
                    nc.vector.tensor_scalar(
                        kc[:, hc, :], psk[:], -CLAMP, CLAMP, ALU.max, ALU.min
                    )

                io_state.append((nch, W, qc, kc, mk_sb, v_sb))

            # ---- A-side factors for both slots ----
            att_all = []
            for s in range(SLOTS):
                nch, W, qc, kc, mk_sb, v_sb = io_state[s]
                # A side hoisted: args/sins/weights for ALL frequencies up
                # front so the scores-matmul LDWEIGHTS never waits on DVE.
                aarg = iop.tile([128, NFREQ, 2, 2 * Q], DT.float32, tag="aarg",
                                name="aarg")
                for j in range(NFREQ):
                    om = float(OMEGA[j])
                    nc.gpsimd.tensor_scalar_mul(
                        aarg[:, j, 0, :], qc[:].rearrange("p c q -> p (c q)"), om)
                    if om * CLAMP > PI:
                        nc.vector.add_range_wrap(
                            aarg[:, j, 0, :], aarg[:, j, 0, :],
                            shift=0.0, bound=PI, period=TWO_PI)
                    nc.vector.tensor_scalar(
                        aarg[:, j, 1, :].bitcast(DT.int32),
                        aarg[:, j, 0, :].bitcast(DT.int32),
                        0x7FFFFFFF, None, ALU.bitwise_and)
                aval = iop.tile([128, NFREQ, 2, 2 * Q], DT.float32, tag="aval",
                                name="aval")
                nc.scalar.activation(aval[:, :, 0, :], aarg[:, :, 0, :], AFT.Sin)
                nc.scalar.activation(aval[:, :, 1, :], aarg[:, :, 1, :], AFT.Sin,
                                     bias=nhpi[:])
                att = iop.tile([128, NFREQ, 2, 2, Q], DT.bfloat16, tag="att",
                               name="att")
                for j in range(NFREQ):
                    for hc in range(2):
                        nc.vector.tensor_scalar_mul(
                            att[:, j, :, hc, :],
                            aval[:, j, :, hc * Q : (hc + 1) * Q].rearrange(
                                "p f q -> p f q"),
                            wvc_sb[:, hc, j : j + 1])

                att_all.append(att)

            # ---- B-side + score matmuls per slot ----
            for s in range(SLOTS):
                nch, W, qc, kc, mk_sb, v_sb = io_state[s]
                att = att_all[s]
                # Scores accumulate into one (64, W) PSUM tile across 2R*2 mms.
                ps_s = psp.tile([Q, W], DT.float32, tag="ps_s", name="ps_s")
                first = True
                for j in range(NFREQ):
                    om = float(OMEGA[j])
                    need_wrap = om * CLAMP > PI

                    # B side: args (128, [fn=2][hc=2][W]) f32
                    barg = workp.tile([128, 2, 2, W], DT.float32, tag="barg",
                                      name="barg")
                    nc.gpsimd.tensor_scalar_mul(
                        barg[:, 0, :, :].rearrange("p c k -> p (c k)"),
                        kc[:].rearrange("p c k -> p (c k)"), om)
                    if need_wrap:
                        nc.vector.add_range_wrap(
                            barg[:, 0, :, :].rearrange("p c k -> p (c k)"),
                            barg[:, 0, :, :].rearrange("p c k -> p (c k)"),
                            shift=0.0, bound=PI, period=TWO_PI)
                    bval = workp.tile([128, 2, 2, W], DT.bfloat16, tag="bval",
                                      name="bval")
                    if j < int(os.environ.get("KERNEL_ACT_ABS", "3")):
                        # balance: |w| on ACT (Abs is in every table set)
                        nc.scalar.activation(
                            barg[:, 1, :, :].rearrange("p c k -> p (c k)"),
                            barg[:, 0, :, :].rearrange("p c k -> p (c k)"),
                            AFT.Abs)
                    else:
                        nc.vector.tensor_scalar(
                            barg[:, 1, :, :].rearrange("p c k -> p (c k)").bitcast(DT.int32),
                            barg[:, 0, :, :].rearrange("p c k -> p (c k)").bitcast(DT.int32),
                            0x7FFFFFFF, None, ALU.bitwise_and)
                    nc.scalar.activation(
                        bval[:, 0, :, :].rearrange("p c k -> p (c k)"),
                        barg[:, 0, :, :].rearrange("p c k -> p (c k)"), AFT.Sin)
                    nc.scalar.activation(
                        bval[:, 1, :, :].rearrange("p c k -> p (c k)"),
                        barg[:, 1, :, :].rearrange("p c k -> p (c k)"), AFT.Sin,
                        bias=nhpi[:])

                    # sin(om(a+b)) = -[sin_a*(-cos_b) + (-cos_a)*sin_b]; the
                    # minus is folded into wvc = -c_j*wv.
                    for hc in range(2):
                        nc.tensor.matmul(
                            ps_s[:], att[:, j, 0, hc, :], bval[:, 1, hc, :],
                            start=first, stop=False)
                        first = False
                        last = j == NFREQ - 1 and hc == 1
                        nc.tensor.matmul(
                            ps_s[:], att[:, j, 1, hc, :], bval[:, 0, hc, :],
                            start=False, stop=last)

                slot_state.append((nch, W, ps_s, mk_sb, v_sb))

            # ---- softmax (Exp) + attn @ V + store, per slot ----
            for s in range(SLOTS):
                nch, W, ps_s, mk_sb, v_sb = slot_state[s]
                sc = smp.tile([Q, W], DT.float32, tag="sc", name="sc")
                nc.vector.tensor_tensor(sc[:], ps_s[:], mk_sb[:], ALU.add)
                p_bf = smp.tile([Q, W], DT.bfloat16, tag="p", name="p_bf")
                S = smp.tile([Q, 1], DT.float32, tag="S", name="S")
                nc.scalar.activation(p_bf[:], sc[:], AFT.Exp, accum_out=S[:])
                sinv = smp.tile([Q, 1], DT.float32, tag="sinv", name="sinv")
                nc.vector.reciprocal_approx_fast(sinv[:], S[:])

                ps_o = psp.tile([Q, DV], DT.float32, tag="ps_o", name="ps_o")
                for c in range(nch):
                    pst = psp.tile([128, Q], DT.bfloat16, tag="pst", name="pst")
                    nc.tensor.transpose(
                        pst[:], p_bf[:, c * 128 : (c + 1) * 128], iden_sb[:])
                    pT = workp.tile([128, Q], DT.bfloat16, tag="pT", name="pT")
                    nc.vector.tensor_copy(pT[:], pst[:])
                    nc.tensor.matmul(
                        ps_o[:], pT[:], v_sb[:, c, :],
                        start=(c == 0), stop=(c == nch - 1),
                    )
                ob = smp.tile([Q, DV], DT.float32, tag="ob", name="ob")
                nc.vector.tensor_scalar_mul(ob[:], ps_o[:], sinv[:])
                nc.sync.dma_start(out_d[s], ob[:])

    nc.compile()
    return nc


def kernel(queries, keys, values, valid_lens, Wq, Wk, wv):
    global LAST_RESULT
    queries = np.asarray(queries, dtype=np.float32)
    keys = np.asarray(keys, dtype=np.float32)
    values = np.asarray(values, dtype=np.float32)
    Wq = np.asarray(Wq, dtype=np.float32)
    Wk = np.asarray(Wk, dtype=np.float32)
    wv = np.asarray(wv, dtype=np.float32)
    vl = np.asarray(valid_lens).astype(np.int64)

    # Per-batch live ki chunk counts; sort so slot 0 takes the 8 largest.
    nch = np.maximum(1, -(-vl // 128)).astype(int)  # ceil(vl/128) in 1..4
    order = np.argsort(-nch, kind="stable")
    slots = [order[:NCORES], order[NCORES:][::-1]]
    nch_slots = tuple(int(nch[sl].max()) for sl in slots)

    nc = _BUILD_CACHE.get(nch_slots)
    if nc is None:
        nc = _build(nch_slots)
        _BUILD_CACHE[nch_slots] = nc

    wq16 = Wq.astype(BF)
    wk16 = Wk.astype(BF)
    wvc = np.empty((128, 2, NFREQ), np.float32)
    for hc in range(2):
        for j in range(NFREQ):
            wvc[:, hc, j] = -float(COEF[j]) * wv[hc * 128 : (hc + 1) * 128]

    ki = np.arange(KV)
    in_maps = []
    for core in range(NCORES):
        qT = np.empty((SLOTS, QS, Q), dtype=BF)
        kT = np.empty((SLOTS, QS, KV), dtype=BF)
        vvv = np.empty((SLOTS, KV, DV), dtype=BF)
        mk = np.empty((SLOTS, Q, KV), dtype=np.float32)
        for s in range(SLOTS):
            b = int(slots[s][core])
            qT[s] = queries[b].T
            kT[s] = keys[b].T
            vvv[s] = values[b]
            mk[s] = np.where(ki < vl[b], 0.0, -1e6)[None, :]
        in_maps.append(
            {"qT": qT, "kT": kT, "vv": vvv, "mk": mk,
             "wq": wq16, "wk": wk16, "wvc": wvc,
             "iden": np.eye(Q, dtype=BF)}
        )

    res = run_bass_kernel_spmd(
        nc,
        in_maps,
        core_ids=list(range(NCORES)),
        trace=bool(os.environ.get("KERNEL_TRACE")),
    )
    LAST_RESULT = res

    out = np.empty((B, Q, DV), dtype=np.float32)
    for core in range(NCORES):
        o = res.results[core]["out"]
        for s in range(SLOTS):
            out[int(slots[s][core])] = o[s]
    return out


# revision 16
# speedup vs baseline: 4.3192x; 4.3192x over previous
"""Additive (Bahdanau) attention TRN2 kernel — 8 NeuronCores, data-parallel.

Math (per batch b):
    qh = queries[b] @ Wq   (Q, H);  kh = keys[b] @ Wk   (KV, H)
    scores[q,k] = sum_h wv[h] * tanh(qh[q,h] + kh[k,h])
    out = softmax(mask(scores)) @ values[b]

Key trick: tanh(x) ~= sum_j c_j sin(om_j x) on [-7, 7] (R=8 harmonics,
sup err ~1e-2, rel contribution ~4e-3), and sin separates over x = a + b:
    sin(om(a+b)) = sin(om a)cos(om b) + cos(om a)sin(om b)
so the (Q, KV, H) tanh tensor is never materialized. Per frequency the
kernel computes sin/cos of the SMALL projected tensors (ACT engine, with
DVE range-reduction into [-pi, pi] via one fused mul pass + at most one
add_range_wrap), then reduces over h with 2R accumulated PE matmuls into
a (64, KV) PSUM scores tile (wv and c_j folded into the q-side weights).
cos is obtained as -cos(w) = sin(|w| - pi/2) so one big ACT Sin per
frequency covers both halves. Projections are clamped to +-3.5 (clamp
error ~1e-4) so one wrap suffices at om_max = 2.69.

Softmax without max-subtraction (scores are O(1)): p = Exp(scores+mask)
with the row sum taken by the same ACT instruction (accum_out); masked
columns give exp(-1e6) = 0. Row normalization is folded into the output
copy. valid_lens sparsity: ki chunks of 128 beyond ceil(valid/128) are
skipped at compile time (per batch-slot, slot-max across cores; batches
sorted so heavy ones share a slot).
"""

import os
import sys

for _p in ("/opt/trn_rl_repo",):
    if os.path.isdir(_p) and _p not in sys.path:
        sys.path.insert(0, _p)

import numpy as np
import ml_dtypes

from concourse import bacc, bass, mybir, tile
from concourse.bass_utils import run_bass_kernel_spmd

BF = ml_dtypes.bfloat16
DT = mybir.dt
AFT = mybir.ActivationFunctionType
ALU = mybir.AluOpType

B, Q, KV, QS, H, DV = 16, 64, 512, 256, 256, 256
NCORES = 8
SLOTS = B // NCORES  # 2 batches per core

CLAMP = 3.5
NFREQ = 8
_L = 9.33
OMEGA = (np.arange(1, NFREQ + 1) * np.pi / _L).astype(np.float64)
_xg = np.linspace(-2 * CLAMP, 2 * CLAMP, 8001)
_A = np.sin(np.outer(_xg, OMEGA))
COEF, *_ = np.linalg.lstsq(_A, np.tanh(_xg), rcond=None)
PI = float(np.pi)
TWO_PI = float(2 * np.pi)
HALF_PI = float(np.pi / 2)

_BUILD_CACHE: dict = {}
LAST_RESULT = None  # BassKernelResults of the most recent run (for test.py)


def _build(nch_slots: tuple) -> "bacc.Bacc":
    nc = bacc.Bacc("TRN2", target_bir_lowering=False, debug=False)

    qT_d = nc.declare_dram_parameter("qT", [SLOTS, QS, Q], DT.bfloat16, isOutput=False)
    kT_d = nc.declare_dram_parameter("kT", [SLOTS, QS, KV], DT.bfloat16, isOutput=False)
    vv_d = nc.declare_dram_parameter("vv", [SLOTS, KV, DV], DT.bfloat16, isOutput=False)
    mk_d = nc.declare_dram_parameter("mk", [SLOTS, Q, KV], DT.float32, isOutput=False)
    wq_d = nc.declare_dram_parameter("wq", [QS, H], DT.bfloat16, isOutput=False)
    wk_d = nc.declare_dram_parameter("wk", [QS, H], DT.bfloat16, isOutput=False)
    wvc_d = nc.declare_dram_parameter("wvc", [128, 2, NFREQ], DT.float32, isOutput=False)
    id_d = nc.declare_dram_parameter("iden", [Q, Q], DT.bfloat16, isOutput=False)
    out_d = nc.declare_dram_parameter("out", [SLOTS, Q, DV], DT.float32, isOutput=True)

    with tile.TileContext(nc) as tc:
        with (
            tc.tile_pool(name="const", bufs=1) as constp,
            tc.tile_pool(name="io", bufs=2) as iop,
            tc.tile_pool(name="work", bufs=3) as workp,
            tc.tile_pool(name="sm", bufs=2) as smp,
            tc.tile_pool(name="ps", bufs=2, space="PSUM") as psp,
        ):
            wq_sb = constp.tile([128, 2, H], DT.bfloat16, name="wq_sb")
            nc.sync.dma_start(wq_sb[:], wq_d.ap().rearrange("(c p) h -> p c h", p=128))
            wk_sb = constp.tile([128, 2, H], DT.bfloat16, name="wk_sb")
            nc.sync.dma_start(wk_sb[:], wk_d.ap().rearrange("(c p) h -> p c h", p=128))
            wvc_sb = constp.tile([128, 2, NFREQ], DT.float32, name="wvc_sb")
            nc.sync.dma_start(wvc_sb[:], wvc_d.ap())
            iden_sb = constp.tile([Q, Q], DT.bfloat16, name="iden_sb")
            nc.sync.dma_start(iden_sb[:], id_d.ap())

            # Pre-warm the sin activation table while input DMAs run.
            _salt = os.environ.get("KERNEL_SALT", "")
            warm = constp.tile([1, 8], DT.float32, name=f"warm{_salt}")
            warm2 = constp.tile([1, 8], DT.float32, name="warm2")
            nc.vector.memset(warm[:], 0.0)
            nc.scalar.activation(warm2[:], warm[:], AFT.Sin)
            nhpi = constp.tile([128, 1], DT.float32, name="nhpi")
            nc.vector.memset(nhpi[:], -HALF_PI)

            slot_state = []
            io_state = []
            # ---- Loads + projections + clamps for both slots ----
            for s in range(SLOTS):
                nch = nch_slots[s]
                W = nch * 128

                qT_sb = iop.tile([128, 2, Q], DT.bfloat16, tag="qT", name="qT_sb")
                nc.scalar.dma_start(
                    qT_sb[:], qT_d[s].rearrange("(c p) q -> p c q", p=128)
                )
                kT_sb = iop.tile([128, 2, W], DT.bfloat16, tag="kT", name="kT_sb")
                for ksc in range(2):
                    nc.sync.dma_start(
                        kT_sb[:, ksc, :],
                        kT_d[s].rearrange("(c p) k -> p c k", p=128)[:, ksc, :W],
                    )
                v_sb = iop.tile([128, nch, DV], DT.bfloat16, tag="v", name="v_sb")
                nc.scalar.dma_start(
                    v_sb[:], vv_d[s].rearrange("(c p) d -> p c d", p=128)[:, :nch]
                )
                mk_sb = iop.tile([Q, W], DT.float32, tag="mk", name="mk_sb")
                nc.gpsimd.dma_start(mk_sb[:], mk_d[s][:, :W])

                # Projections (PSUM f32) then clamp to +-CLAMP into SBUF f32.
                qc = iop.tile([128, 2, Q], DT.float32, tag="qc", name="qc")
                kc = iop.tile([128, 2, W], DT.float32, tag="kc", name="kc")
                for hc in range(2):
                    hsl = slice(hc * 128, (hc + 1) * 128)
                    psq = psp.tile([128, Q], DT.float32, tag="pproj", name="psq")
                    for ksc in range(2):
                        nc.tensor.matmul(
                            psq[:],
                            wq_sb[:, ksc, hsl],
                            qT_sb[:, ksc, :],
                            start=(ksc == 0),
                            stop=(ksc == 1),
                        )
                    nc.vector.tensor_scalar(
                        qc[:, hc, :], psq[:], -CLAMP, CLAMP, ALU.max, ALU.min
                    )
                    psk = psp.tile([128, W], DT.float32, tag="pproj", name="psk")
                    for ksc in range(2):
                        nc.tensor.matmul(
                            psk[:],
                            wk_sb[:, ksc, hsl],
                            kT_sb[:, ksc, :],
                            start=(ksc == 0),
                            stop=(ksc == 1),
                        )
                    nc.vector.tensor_scalar(
                        kc[:, hc, :], psk[:], -CLAMP, CLAMP, ALU.max, ALU.min
                    )

                io_state.append((nch, W, qc, kc, mk_sb, v_sb))

            # ---- A-side factors for both slots ----
            att_all = []
            for s in range(SLOTS):
                nch, W, qc, kc, mk_sb, v_sb = io_state[s]
                # A side hoisted: args/sins/weights for ALL frequencies up
                # front so the scores-matmul LDWEIGHTS never waits on DVE.
                aarg = iop.tile([128, NFREQ, 2, 2 * Q], DT.float32, tag="aarg",
                                name="aarg")
                for j in range(NFREQ):
                    om = float(OMEGA[j])
                    nc.vector.tensor_scalar_mul(
                        aarg[:, j, 0, :], qc[:].rearrange("p c q -> p (c q)"), om)
                    if om * CLAMP > PI:
                        nc.vector.add_range_wrap(
                            aarg[:, j, 0, :], aarg[:, j, 0, :],
                            shift=0.0, bound=PI, period=TWO_PI)
                    nc.vector.tensor_scalar(
                        aarg[:, j, 1, :].bitcast(DT.int32),
                        aarg[:, j, 0, :].bitcast(DT.int32),
                        0x7FFFFFFF, None, ALU.bitwise_and)
                aval = iop.tile([128, NFREQ, 2, 2 * Q], DT.float32, tag="aval",
                                name="aval")
                nc.scalar.activation(aval[:, :, 0, :], aarg[:, :, 0, :], AFT.Sin)
                nc.scalar.activation(aval[:, :, 1, :], aarg[:, :, 1, :], AFT.Sin,
                                     bias=nhpi[:])
                att = iop.tile([128, NFREQ, 2, 2, Q], DT.bfloat16, tag="att",
                               name="att")
                for j in range(NFREQ):
                    for hc in range(2):
                        nc.vector.tensor_scalar_mul(
                            att[:, j, :, hc, :],
                            aval[:, j, :, hc * Q : (hc + 1) * Q].rearrange(
                                "p f q -> p f q"),
                            wvc_sb[:, hc, j : j + 1])

                att_all.append(att)

            # ---- B-side + score matmuls per slot ----
            for s in range(SLOTS):
                nch, W, qc, kc, mk_sb, v_sb = io_state[s]
                att = att_all[s]
                # Scores accumulate into one (64, W) PSUM tile across 2R*2 mms.
                ps_s = psp.tile([Q, W], DT.float32, tag="ps_s", name="ps_s")
                first = True
                for j in range(NFREQ):
                    om = float(OMEGA[j])
                    need_wrap = om * CLAMP > PI

                    # B side: args (128, [fn=2][hc=2][W]) f32
                    barg = workp.tile([128, 2, 2, W], DT.float32, tag="barg",
                                      name="barg")
                    nc.vector.tensor_scalar_mul(
                        barg[:, 0, :, :].rearrange("p c k -> p (c k)"),
                        kc[:].rearrange("p c k -> p (c k)"), om)
                    if need_wrap:
                        nc.vector.add_range_wrap(
                            barg[:, 0, :, :].rearrange("p c k -> p (c k)"),
                            barg[:, 0, :, :].rearrange("p c k -> p (c k)"),
                            shift=0.0, bound=PI, period=TWO_PI)
                    bval = workp.tile([128, 2, 2, W], DT.bfloat16, tag="bval",
                                      name="bval")
                    if j < int(os.environ.get("KERNEL_ACT_ABS", "3")):
                        # balance: |w| on ACT (Abs is in every table set)
                        nc.scalar.activation(
                            barg[:, 1, :, :].rearrange("p c k -> p (c k)"),
                            barg[:, 0, :, :].rearrange("p c k -> p (c k)"),
                            AFT.Abs)
                    else:
                        nc.vector.tensor_scalar(
                            barg[:, 1, :, :].rearrange("p c k -> p (c k)").bitcast(DT.int32),
                            barg[:, 0, :, :].rearrange("p c k -> p (c k)").bitcast(DT.int32),
                            0x7FFFFFFF, None, ALU.bitwise_and)
                    nc.scalar.activation(
                        bval[:, 0, :, :].rearrange("p c k -> p (c k)"),
                        barg[:, 0, :, :].rearrange("p c k -> p (c k)"), AFT.Sin)
                    nc.scalar.activation(
                        bval[:, 1, :, :].rearrange("p c k -> p (c k)"),
                        barg[:, 1, :, :].rearrange("p c k -> p (c k)"), AFT.Sin,
                        bias=nhpi[:])

                    # sin(om(a+b)) = -[sin_a*(-cos_b) + (-cos_a)*sin_b]; the
                    # minus is folded into wvc = -c_j*wv.
                    for hc in range(2):
                        nc.tensor.matmul(
                            ps_s[:], att[:, j, 0, hc, :], bval[:, 1, hc, :],
                            start=first, stop=False)
                        first = False
                        last = j == NFREQ - 1 and hc == 1
                        nc.tensor.matmul(
                            ps_s[:], att[:, j, 1, hc, :], bval[:, 0, hc, :],
                            start=False, stop=last)

                slot_state.append((nch, W, ps_s, mk_sb, v_sb))

            # ---- softmax (Exp) + attn @ V + store, per slot ----
            for s in range(SLOTS):
                nch, W, ps_s, mk_sb, v_sb = slot_state[s]
                sc = smp.tile([Q, W], DT.float32, tag="sc", name="sc")
                nc.vector.tensor_tensor(sc[:], ps_s[:], mk_sb[:], ALU.add)
                p_bf = smp.tile([Q, W], DT.bfloat16, tag="p", name="p_bf")
                S = smp.tile([Q, 1], DT.float32, tag="S", name="S")
                nc.scalar.activation(p_bf[:], sc[:], AFT.Exp, accum_out=S[:])
                sinv = smp.tile([Q, 1], DT.float32, tag="sinv", name="sinv")
                nc.vector.reciprocal_approx_fast(sinv[:], S[:])

                ps_o = psp.tile([Q, DV], DT.float32, tag="ps_o", name="ps_o")
                for c in range(nch):
                    pst = psp.tile([128, Q], DT.bfloat16, tag="pst", name="pst")
                    nc.tensor.transpose(
                        pst[:], p_bf[:, c * 128 : (c + 1) * 128], iden_sb[:])
                    pT = workp.tile([128, Q], DT.bfloat16, tag="pT", name="pT")
                    nc.vector.tensor_copy(pT[:], pst[:])
                    nc.tensor.matmul(
                        ps_o[:], pT[:], v_sb[:, c, :],
                        start=(c == 0), stop=(c == nch - 1),
                    )
                ob = smp.tile([Q, DV], DT.float32, tag="ob", name="ob")
                nc.vector.tensor_scalar_mul(ob[:], ps_o[:], sinv[:])
                nc.sync.dma_start(out_d[s], ob[:])

    nc.compile()
    return nc


def kernel(queries, keys, values, valid_lens, Wq, Wk, wv):
    global LAST_RESULT
    queries = np.asarray(queries, dtype=np.float32)
    keys = np.asarray(keys, dtype=np.float32)
    values = np.asarray(values, dtype=np.float32)
    Wq = np.asarray(Wq, dtype=np.float32)
    Wk = np.asarray(Wk, dtype=np.float32)
    wv = np.asarray(wv, dtype=np.float32)
    vl = np.asarray(valid_lens).astype(np.int64)

    # Per-batch live ki chunk counts; sort so slot 0 takes the 8 largest.
    nch = np.maximum(1, -(-vl // 128)).astype(int)  # ceil(vl/128) in 1..4
    order = np.argsort(-nch, kind="stable")
    slots = [order[:NCORES], order[NCORES:][::-1]]
    nch_slots = tuple(int(nch[sl].max()) for sl in slots)

    nc = _BUILD_CACHE.get(nch_slots)
    if nc is None:
        nc = _build(nch_slots)
        _BUILD_CACHE[nch_slots] = nc

    wq16 = Wq.astype(BF)
    wk16 = Wk.astype(BF)
    wvc = np.empty((128, 2, NFREQ), np.float32)
    for hc in range(2):
        for j in range(NFREQ):
            wvc[:, hc, j] = -float(COEF[j]) * wv[hc * 128 : (hc + 1) * 128]

    ki = np.arange(KV)
    in_maps = []
    for core in range(NCORES):
        qT = np.empty((SLOTS, QS, Q), dtype=BF)
        kT = np.empty((SLOTS, QS, KV), dtype=BF)
        vvv = np.empty((SLOTS, KV, DV), dtype=BF)
        mk = np.empty((SLOTS, Q, KV), dtype=np.float32)
        for s in range(SLOTS):
            b = int(slots[s][core])
            qT[s] = queries[b].T
            kT[s] = keys[b].T
            vvv[s] = values[b]
            mk[s] = np.where(ki < vl[b], 0.0, -1e6)[None, :]
        in_maps.append(
            {"qT": qT, "kT": kT, "vv": vvv, "mk": mk,
             "wq": wq16, "wk": wk16, "wvc": wvc,
             "iden": np.eye(Q, dtype=BF)}
        )

    res = run_bass_kernel_spmd(
        nc,
        in_maps,
        core_ids=list(range(NCORES)),
        trace=bool(os.environ.get("KERNEL_TRACE")),
    )
    LAST_RESULT = res

    out = np.empty((B, Q, DV), dtype=np.float32)
    for core in range(NCORES):
        o = res.results[core]["out"]
        for s in range(SLOTS):
            out[int(slots[s][core])] = o[s]
    return out
